# revision 3
# baseline (speedup 1.0000x reference)
"""GRU free-run greedy decoder on 8 Trainium2 NeuronCores (data parallel), v9.

All-fp16 compensated 3-pass scheme (replay: 0 flipped tokens in all
65536 decode positions; dropping any pass flips 23-211 -> fails 2e-2):
    pass1 fp16: h11 @ W11            (h11 = fp16(h), W11 = fp16(W))
    pass2 fp16: hr  @ W11            (hr = fp16(h - h11), exact)
    pass3 fp16: (h*2^-6) @ (Wr*2^6)  (Wr = fp16(W - W11))
dropping only hr*Wr ~ 2^-22. fp16 operands everywhere: LDWEIGHTS ~95ns
(vs 190 for f32r) and half the weight-DMA startup.

No PSUM bias-seed matmuls: biases/Lc are added by per-bank DVE adds the
moment each PSUM accumulation bank closes (fc bias stays a single PE
seed; its DVE add sat on the latency-critical argmax boundary chain).
PE work is ordered so every serial chain is covered by independent
matmul streams: emb/gi1 output chunks issue j0 -> n -> j1 so the gate
chain starts at the first bank close; gh1_n passes and the gh0_n
prefetch sit right after each transpose_split to cover the ACT-cast ->
LDWEIGHTS latency of the fresh h^T stationaries; h-state updates run on
DVE (GpSimd tensor ops are ~2x slower and sat on the chain).
"""

import sys
import numpy as np

sys.path.insert(0, "/opt/trn_rl_repo")

P = 128          # partitions == per-core batch
H = 512          # hidden
V = 256          # vocab
LAT = 256        # latent dim
G3 = 3 * H       # 1536 gate width
T_FULL = 64
N_CORES = 8

_CACHE = {}


def build_program(T=T_FULL):
    import concourse.bass as bass
    import concourse.tile as tile
    from concourse import bacc, mybir
    from concourse.masks import make_identity

    f32 = mybir.dt.float32
    f32r = mybir.dt.float32r
    f16 = mybir.dt.float16
    bf16 = mybir.dt.bfloat16
    AF = mybir.ActivationFunctionType
    OP = mybir.AluOpType
    ts = bass.ts

    nc = bacc.Bacc(
        "TRN2", target_bir_lowering=False, debug=False,
        enable_asserts=False, num_devices=N_CORES,
    )

    # ---- DRAM I/O ----
    wembT_d = nc.dram_tensor("wembT", [2, P, G3], f16, kind="ExternalInput").ap()
    wembr_d = nc.dram_tensor("wembr", [2, P, G3], f16, kind="ExternalInput").ap()
    whh0T_d = nc.dram_tensor("whh0T", [4, P, G3], f16, kind="ExternalInput").ap()
    wih1T_d = nc.dram_tensor("wih1T", [4, P, G3], f16, kind="ExternalInput").ap()
    whh1T_d = nc.dram_tensor("whh1T", [4, P, G3], f16, kind="ExternalInput").ap()
    whh0r_d = nc.dram_tensor("whh0r", [4, P, G3], f16, kind="ExternalInput").ap()
    wih1r_d = nc.dram_tensor("wih1r", [4, P, G3], f16, kind="ExternalInput").ap()
    whh1r_d = nc.dram_tensor("whh1r", [4, P, G3], f16, kind="ExternalInput").ap()
    wfcT_d = nc.dram_tensor("wfcT", [4, P, V], f16, kind="ExternalInput").ap()
    wfcr_d = nc.dram_tensor("wfcr", [4, P, V], f16, kind="ExternalInput").ap()
    # bias tensors added by DVE post-PSUM (per-row Lc + replicated consts)
    lcrz_d = nc.dram_tensor("lcrz", [P, 2 * H], f32, kind="ExternalInput").ap()
    lcn_d = nc.dram_tensor("lcn", [P, H], f32, kind="ExternalInput").ap()
    b0hn_d = nc.dram_tensor("b0hn", [P, H], f32, kind="ExternalInput").ap()
    b1rz_d = nc.dram_tensor("b1rz", [P, 2 * H], f32, kind="ExternalInput").ap()
    b1in_d = nc.dram_tensor("b1in", [P, H], f32, kind="ExternalInput").ap()
    b1hn_d = nc.dram_tensor("b1hn", [P, H], f32, kind="ExternalInput").ap()
    # fc bias stays a PE seed (exact 3-way bf16 stack)
    bfcs_d = nc.dram_tensor("bfcs", [3, V], bf16, kind="ExternalInput").ap()
    out_d = nc.dram_tensor("out", [P, T, V], f32, kind="ExternalOutput").ap()

    from contextlib import ExitStack
    with tile.TileContext(nc) as tc, ExitStack() as ctx:
        wt = ctx.enter_context(tc.tile_pool(name="wt", bufs=1))
        st = ctx.enter_context(tc.tile_pool(name="st", bufs=1))
        wk = ctx.enter_context(tc.tile_pool(name="wk", bufs=1))
        ps = ctx.enter_context(tc.tile_pool(name="ps", bufs=2, space="PSUM"))
        ps1 = ctx.enter_context(tc.tile_pool(name="ps1", bufs=1, space="PSUM"))

        # ---- persistent weights/biases in SBUF ----
        whh0T = wt.tile([P, 4, G3], f16, tag="whh0T")
        wih1T = wt.tile([P, 4, G3], f16, tag="wih1T")
        whh1T = wt.tile([P, 4, G3], f16, tag="whh1T")
        whh0r = wt.tile([P, 4, G3], f16, tag="whh0r")
        wih1r = wt.tile([P, 4, G3], f16, tag="wih1r")
        whh1r = wt.tile([P, 4, G3], f16, tag="whh1r")
        wembT = wt.tile([P, 2, G3], f16, tag="wembT")
        wembr = wt.tile([P, 2, G3], f16, tag="wembr")
        wfcT = wt.tile([P, 4, V], f16, tag="wfcT")
        wfcr = wt.tile([P, 4, V], f16, tag="wfcr")

        # DMA order == first-use order; tiny bias tensors first so the
        # t=0 gate chain never waits behind ~17MB of weights.
        lcrz = wt.tile([P, 2 * H], f32, tag="lcrz")
        lcn = wt.tile([P, H], f32, tag="lcn")
        b0hn = wt.tile([P, H], f32, tag="b0hn")
        b1rz = wt.tile([P, 2 * H], f32, tag="b1rz")
        b1in = wt.tile([P, H], f32, tag="b1in")
        b1hn = wt.tile([P, H], f32, tag="b1hn")
        bfcs = wt.tile([3, V], bf16, tag="bfcs")
        nc.sync.dma_start(lcrz[:], lcrz_d[:])
        nc.sync.dma_start(lcn[:], lcn_d[:])
        nc.sync.dma_start(b0hn[:], b0hn_d[:])
        nc.sync.dma_start(b1rz[:], b1rz_d[:])
        nc.sync.dma_start(b1in[:], b1in_d[:])
        nc.sync.dma_start(b1hn[:], b1hn_d[:])
        nc.sync.dma_start(bfcs[:], bfcs_d[:])
        for kc in range(4):
            nc.sync.dma_start(wih1T[:, kc, :], wih1T_d[kc])
            nc.sync.dma_start(wih1r[:, kc, :], wih1r_d[kc])
        for kc in range(4):
            nc.sync.dma_start(whh0T[:, kc, :], whh0T_d[kc])
            nc.sync.dma_start(whh0r[:, kc, :], whh0r_d[kc])
        for kc in range(4):
            nc.sync.dma_start(wfcT[:, kc, :], wfcT_d[kc])
            nc.sync.dma_start(wfcr[:, kc, :], wfcr_d[kc])
        for kc in range(4):
            nc.sync.dma_start(whh1T[:, kc, :], whh1T_d[kc])
            nc.sync.dma_start(whh1r[:, kc, :], whh1r_d[kc])
        for kc in range(2):
            nc.sync.dma_start(wembT[:, kc, :], wembT_d[kc])
            nc.sync.dma_start(wembr[:, kc, :], wembr_d[kc])

        ones3 = wt.tile([3, P], bf16, tag="ones3")
        nc.gpsimd.memset(ones3[:], 1.0)
        zer = wt.tile([P, H], bf16, tag="zer")
        nc.gpsimd.memset(zer[:], 0.0)
        identb = wt.tile([P, P], bf16, tag="identb")
        make_identity(nc, identb[:])
        ident = wt.tile([P, P], f32, tag="ident")
        make_identity(nc, ident[:])

        # ---- persistent state ----
        h0 = st.tile([P, H], f32, tag="h0")
        h1 = st.tile([P, H], f32, tag="h1")
        nc.gpsimd.memset(h0[:], 0.0)
        nc.gpsimd.memset(h1[:], 0.0)
        h0T = st.tile([P, 4, P], f16, tag="h0T")     # RN12(h0)^T
        h0rT = st.tile([P, 4, P], f16, tag="h0rT")   # residual^T
        h0mT = st.tile([P, 4, P], f16, tag="h0mT")    # fp16(h0^T * 2^-6)
        h1T = st.tile([P, 4, P], f16, tag="h1T")
        h1rT = st.tile([P, 4, P], f16, tag="h1rT")
        h1mT = st.tile([P, 4, P], f16, tag="h1mT")
        ohT = st.tile([P, 2, P], f16, tag="ohT")
        ohTm = st.tile([P, 2, P], f16, tag="ohTm")    # one-hot^T * 2^-6

        def mm3(dest, hT, hrT, hmT, w, wr, col, width, start=False,
                stop=False):
            """3-pass compensated-f32r accumulation of h @ W[:, col:col+width]."""
            for kc in range(4):
                nc.tensor.matmul(dest, hT[:, kc, :], w[:, kc, col:col + width],
                                 start=(start and kc == 0), stop=False)
            for kc in range(4):
                nc.tensor.matmul(dest, hrT[:, kc, :], w[:, kc, col:col + width],
                                 start=False, stop=False)
            for kc in range(4):
                nc.tensor.matmul(dest, hmT[:, kc, :], wr[:, kc, col:col + width],
                                 start=False, stop=(stop and kc == 3))

        def gates_front(rzpre, tag):
            """sigmoids of r, z (ACT)."""
            rr = wk.tile([P, H], f32, tag="rr", name=f"rr{tag}")
            nc.scalar.activation(rr[:], rzpre[:, 0:512], AF.Sigmoid)
            zz = wk.tile([P, H], f32, tag="zz", name=f"zz{tag}")
            nc.scalar.activation(zz[:], rzpre[:, 512:1024], AF.Sigmoid)
            return rr, zz

        def gates_back(rr, zz, inb, hnb, h, tag):
            """n gate + state update; h updated in place.
            h' = n + z*(h - n), all three update ops on DVE (baseline used
            GpSimd for two of them; GpSimd TT is ~1.3us vs DVE 0.69)."""
            rhn = wk.tile([P, H], f32, tag="rhn", name=f"rhn{tag}")
            nc.vector.tensor_mul(rhn[:], rr[:], hnb)
            npre = wk.tile([P, H], f32, tag="npre", name=f"npre{tag}")
            nc.vector.tensor_add(npre[:], inb, rhn[:])
            nn = wk.tile([P, H], f32, tag="nn", name=f"nn{tag}")
            nc.scalar.activation(nn[:], npre[:], AF.Tanh)
            dd = wk.tile([P, H], f32, tag="dd", name=f"dd{tag}")
            nc.vector.tensor_sub(dd[:], h[:], nn[:])
            zd = wk.tile([P, H], f32, tag="zd", name=f"zd{tag}")
            nc.vector.tensor_mul(zd[:], zz[:], dd[:])
            nc.vector.tensor_add(h[:], nn[:], zd[:])

        def transpose_split(h, hT, hrT, hmT, tail):
            """h [P,512] -> fp32 h^T in tail psum [0:512]; then
            hT = f32r cast on ACT (rounds), hrT = psum - hT (DVE),
            hmT = fp16(psum*2^-6) (ACT)."""
            for kc in range(4):
                nc.tensor.transpose(tail[:, ts(kc, P)], h[:, ts(kc, P)], ident[:])
            tsl = tail[:, 0:512]
            nc.scalar.copy(hT[:, :, :].rearrange("p a b -> p (a b)"), tsl)
            nc.vector.tensor_sub(hrT[:, :, :].rearrange("p a b -> p (a b)"),
                                 tsl, hT[:, :, :].rearrange("p a b -> p (a b)"))
            nc.scalar.mul(hmT[:, :, :].rearrange("p a b -> p (a b)"), tsl, 2.0 ** -6)

        def argmax_tail(t, tail, lg):
            """argmax(lg psum, fc bias already seeded) -> one-hot -> DMA +
            ohT/ohTm (tail 256:512)."""
            mx = wk.tile([P, 1], f32, tag="mx", name=f"mx_{t}")
            nc.vector.reduce_max(mx[:], lg, axis=mybir.AxisListType.X)
            oh = wk.tile([P, V], f32, tag="oh", name=f"oh_{t}")
            nc.vector.tensor_scalar(oh[:], lg, mx[:, 0:1], None,
                                    op0=OP.is_equal)
            nc.sync.dma_start(out_d[:, t, :], oh[:])
            if tail is not None:
                for v in range(2):
                    nc.tensor.transpose(tail[:, 256 + v * P:256 + (v + 1) * P],
                                        oh[:, ts(v, P)], ident[:])
                tsl = tail[:, 256:512]
                nc.scalar.copy(
                    ohT[:, :, :].rearrange("p a b -> p (a b)"), tsl)
                nc.scalar.mul(
                    ohTm[:, :, :].rearrange("p a b -> p (a b)"), tsl, 2.0 ** -6)

        # ---- the T decode steps ----
        prev_tail = None
        prev_lg = None
        ng0rz = ng0ihn = None
        for t in range(T):
            g0rz, g0ihn = ng0rz, ng0ihn  # gh0 part prefetched at t-1 (None at t=0)

            # -- step t-1 tail first: argmax -> one-hot -> ohT. gh1_rz
            # pass1 fills the PE while the DVE computes the argmax; it also
            # OPENS the g1rz group (start=True). --
            if t > 0:
                g1rz = ps.tile([P, 1024], f32, tag="rz", name=f"g1rz_{t}")
                for j in (0, 1):  # gh1_rz pass1: boundary filler
                    for kc in range(4):
                        nc.tensor.matmul(g1rz[:, ts(j, 512)], h1T[:, kc, :],
                                         whh1T[:, kc, ts(j, 512)],
                                         start=(kc == 0), stop=False)
                argmax_tail(t - 1, prev_tail, prev_lg)

                # l0 h_n bias add: its psum bank closed at t-1's prefetch,
                # so it runs during the boundary.
                hn0b = wk.tile([P, H], f32, tag="hnp", name=f"hn0p_{t}")
                nc.vector.tensor_add(hn0b[:], g0ihn[:, 512:1024], b0hn[:])

            # -- emb, ordered to start the l0 gate chain as early as
            # possible: rz f32r (j0, j1), then f16 j0 (closes the r|z j0
            # bank -> rz0 r-add -> sigmoid_r), then emb_n f32r, f16 j1,
            # emb_n f16. wembT passes come first so the PE never waits on
            # the ohTm scale-copy. --
            if t > 0:
                rz0p = wk.tile([P, 1024], f32, tag="rzp", name=f"rz0p_{t}")
                in0b = wk.tile([P, H], f32, tag="inp", name=f"in0p_{t}")
                for j in (0, 1):
                    for v in range(2):
                        nc.tensor.matmul(g0rz[:, ts(j, 512)], ohT[:, v, :],
                                         wembT[:, v, ts(j, 512)],
                                         start=False, stop=False)
                for v in range(2):
                    nc.tensor.matmul(g0rz[:, 0:512], ohTm[:, v, :],
                                     wembr[:, v, 0:512],
                                     start=False, stop=(v == 1))
                nc.vector.tensor_add(rz0p[:, 0:512], g0rz[:, 0:512],
                                     lcrz[:, 0:512])
                for v in range(2):
                    nc.tensor.matmul(g0ihn[:, 0:512], ohT[:, v, :],
                                     wembT[:, v, 1024:1536],
                                     start=(v == 0), stop=False)
                for v in range(2):
                    nc.tensor.matmul(g0rz[:, 512:1024], ohTm[:, v, :],
                                     wembr[:, v, 512:1024],
                                     start=False, stop=(v == 1))
                nc.vector.tensor_add(rz0p[:, 512:1024], g0rz[:, 512:1024],
                                     lcrz[:, 512:1024])
                for v in range(2):
                    nc.tensor.matmul(g0ihn[:, 0:512], ohTm[:, v, :],
                                     wembr[:, v, 1024:1536],
                                     start=False, stop=(v == 1))
                nc.vector.tensor_add(in0b[:], g0ihn[:, 0:512], lcn[:])

            if t > 0:
                # gh1 rz residual passes; pass1 ran at the step boundary.
                # Overlaps the l0 gate chain.
                for j in (0, 1):
                    for kc in range(4):
                        nc.tensor.matmul(g1rz[:, ts(j, 512)], h1rT[:, kc, :],
                                         whh1T[:, kc, ts(j, 512)],
                                         start=False, stop=False)
                    for kc in range(4):
                        nc.tensor.matmul(g1rz[:, ts(j, 512)], h1mT[:, kc, :],
                                         whh1r[:, kc, ts(j, 512)],
                                         start=False, stop=False)

            # -- layer0 gates -> h0 --
            if t > 0:
                rr0, zz0 = gates_front(rz0p[:], f"0_{t}")
                gates_back(rr0, zz0, in0b[:], hn0b[:], h0, f"0_{t}")
            else:
                # t=0: h0 == 0 and one-hot == 0 -> preactivations are the
                # pure bias tensors straight from SBUF.
                rr0, zz0 = gates_front(lcrz[:], "0_0")
                gates_back(rr0, zz0, lcn[:], b0hn[:], h0, "0_0")

            # -- gh1_n passes 1+2: PE filler while the DVE finishes the l0
            # gate chain; OPENS the g1ihn h_n region --
            g1ihn = ps1.tile([P, 1024], f32, tag="ihn", name=f"g1ihn_{t}")
            if t > 0:
                for kc in range(4):
                    nc.tensor.matmul(g1ihn[:, 512:1024], h1T[:, kc, :],
                                     whh1T[:, kc, 1024:1536],
                                     start=(kc == 0), stop=False)
                for kc in range(4):
                    nc.tensor.matmul(g1ihn[:, 512:1024], h1rT[:, kc, :],
                                     whh1T[:, kc, 1024:1536],
                                     start=False, stop=False)

            # -- h0'^T -> h0T / h0rT / h0mT --
            tail = ps1.tile([P, 1024], f32, tag="tail", name=f"tail_{t}")
            transpose_split(h0, h0T, h0rT, h0mT, tail)

            # -- gh1_n pass 3: fills the PE while the ACT cast produces
            # h0T (the gi1 matmuls' stationary) --
            if t > 0:
                for kc in range(4):
                    nc.tensor.matmul(g1ihn[:, 512:1024], h1mT[:, kc, :],
                                     whh1r[:, kc, 1024:1536],
                                     start=False, stop=(kc == 3))
                hn1b = wk.tile([P, H], f32, tag="hnp", name=f"hn1p_{t}")
                nc.vector.tensor_add(hn1b[:], g1ihn[:, 512:1024], b1hn[:])

            # -- gi1 (= h0' @ Wih1T): rz j0 first (shortest path to the l1
            # sigmoid_r), then i_n, then rz j1; per-bank bias adds issue as
            # each bank stops. At t=0 gi1 opens the g1rz group. --
            if t == 0:
                g1rz = ps.tile([P, 1024], f32, tag="rz", name="g1rz_0")
            rz1p = wk.tile([P, 1024], f32, tag="rzp", name=f"rz1p_{t}")
            in1b = wk.tile([P, H], f32, tag="inp", name=f"in1p_{t}")
            mm3(g1rz[:, 0:512], h0T, h0rT, h0mT, wih1T, wih1r,
                0, 512, start=(t == 0), stop=True)
            nc.vector.tensor_add(rz1p[:, 0:512], g1rz[:, 0:512],
                                 b1rz[:, 0:512])
            mm3(g1ihn[:, 0:512], h0T, h0rT, h0mT, wih1T, wih1r, 1024, 512,
                start=True, stop=True)
            nc.vector.tensor_add(in1b[:], g1ihn[:, 0:512], b1in[:])
            mm3(g1rz[:, 512:1024], h0T, h0rT, h0mT, wih1T, wih1r,
                512, 512, start=(t == 0), stop=True)
            nc.vector.tensor_add(rz1p[:, 512:1024], g1rz[:, 512:1024],
                                 b1rz[:, 512:1024])

            # -- next step's gh0_rz: h0T fresh; PE chews through these
            # during the whole l1 gate chain. --
            if t + 1 < T:
                ng0rz = ps.tile([P, 1024], f32, tag="rz", name=f"g0rz_{t+1}")
                for j in range(2):
                    mm3(ng0rz[:, ts(j, 512)], h0T, h0rT, h0mT, whh0T, whh0r,
                        j * 512, 512, start=True)
            else:
                # t == T-1: no prefetch work exists; keep the in-order PE
                # queue from running dry during the last l1 gate chain.
                for _ in range(22):
                    nc.tensor.matmul(tail[:, 256:512], identb[:],
                                     zer[:, 0:256], start=True, stop=True)

            # -- layer1 gates -> h1 --
            rr1, zz1 = gates_front(rz1p[:], f"1_{t}")
            gates_back(rr1, zz1, in1b[:],
                       hn1b[:] if t > 0 else b1hn[:], h1, f"1_{t}")

            # -- h1'^T --
            transpose_split(h1, h1T, h1rT, h1mT, tail)

            # -- next step's gh0_n: h0-dependent, fills the PE while the
            # ACT cast produces h1T (the fc matmuls' stationary) --
            if t + 1 < T:
                ng0ihn = ps1.tile([P, 1024], f32, tag="ihn",
                                  name=f"g0ihn_{t+1}")
                mm3(ng0ihn[:, 512:1024], h0T, h0rT, h0mT, whh0T, whh0r,
                    1024, 512, start=True, stop=True)

            # -- fc logits (+bias seed) -> tail cols [0:256] --
            nc.tensor.matmul(tail[:, 0:256], ones3[:], bfcs[:, 0:256],
                             start=True, stop=False)
            for kc in range(4):
                nc.tensor.matmul(tail[:, 0:256], h1T[:, kc, :],
                                 wfcT[:, kc, :], start=False, stop=False)
            for kc in range(4):
                nc.tensor.matmul(tail[:, 0:256], h1rT[:, kc, :],
                                 wfcT[:, kc, :], start=False, stop=False)
            for kc in range(4):
                nc.tensor.matmul(tail[:, 0:256], h1mT[:, kc, :],
                                 wfcr[:, kc, :], start=False, stop=(kc == 3))
            prev_tail, prev_lg = tail, tail[:, 0:256]

        argmax_tail(T - 1, None, prev_lg)

    nc.compile()
    return nc


def _rn12_even(a):
    """RN-ties-even to 12 mantissa bits (11 explicit) — matches TRN2 f32r."""
    u = np.ascontiguousarray(a.astype(np.float32)).view(np.uint32).copy()
    sign = u & np.uint32(0x80000000)
    mag = u & np.uint32(0x7FFFFFFF)
    mag = (mag + np.uint32(0x7FF) + ((mag >> np.uint32(12)) & np.uint32(1))) \
        & np.uint32(0xFFFFF000)
    return (sign | mag).view(np.float32)


def _split3_bf16(a):
    """EXACT 3-way bf16 split: a == hi + mid + lo in fp32 (any add order)."""
    import ml_dtypes
    f4, bf = np.float32, ml_dtypes.bfloat16
    a = a.astype(f4)
    hi = a.astype(bf)
    r = a - hi.astype(f4)
    mid = r.astype(bf)
    lo = (r - mid.astype(f4)).astype(bf)
    return np.ascontiguousarray(np.stack([hi, mid, lo]))


def prep_host_inputs(latent_vec, w_ih0, w_hh0, b_ih0, b_hh0,
                     w_ih_r, w_hh_r, b_ih_r, b_hh_r, w_fc, b_fc):
    """Host prep: transposes/reshapes, RN12 weight splits, Lc fold."""
    f4 = np.float32
    f2 = np.float16

    def wsplit(wT, kchunks):
        w = np.ascontiguousarray(wT.astype(f4)).reshape(kchunks, P, -1)
        w16 = np.ascontiguousarray(w.astype(f2))
        wr = np.ascontiguousarray(((w - w16.astype(f4)) * 64.0).astype(f2))
        return w16, wr

    wembT, wembr = wsplit(w_ih0[:, LAT:].T, 2)
    whh0T, whh0r = wsplit(w_hh0.T, 4)
    wih1T, wih1r = wsplit(w_ih_r[0].T, 4)
    whh1T, whh1r = wsplit(w_hh_r[0].T, 4)
    wfcT, wfcr = wsplit(w_fc.T, 4)

    # Lc = latent @ W_lat^T + b_ih0 (+ b_hh0 on the rz part): constant
    # across steps; computed here once in fp32 (pure input transform).
    Lc = latent_vec.astype(f4) @ w_ih0[:, :LAT].astype(f4).T + b_ih0.astype(f4)
    Lc[:, :1024] += b_hh0[:1024].astype(f4)

    def rep(v):
        return np.ascontiguousarray(
            np.broadcast_to(v.astype(f4), (P, v.shape[0])))

    common = dict(
        wembT=wembT, wembr=wembr,
        whh0T=whh0T, wih1T=wih1T, whh1T=whh1T,
        whh0r=whh0r, wih1r=wih1r, whh1r=whh1r,
        wfcT=wfcT, wfcr=wfcr,
        b0hn=rep(b_hh0[1024:]),
        b1rz=rep(b_ih_r[0][:1024] + b_hh_r[0][:1024]),
        b1in=rep(b_ih_r[0][1024:]),
        b1hn=rep(b_hh_r[0][1024:]),
        bfcs=_split3_bf16(b_fc),
    )
    in_maps = []
    for c in range(N_CORES):
        m = dict(common)
        lc_c = Lc[c * P:(c + 1) * P]
        m["lcrz"] = np.ascontiguousarray(lc_c[:, :1024])
        m["lcn"] = np.ascontiguousarray(lc_c[:, 1024:])
        in_maps.append(m)
    return in_maps


def kernel(**inputs):
    from concourse import bass_utils

    key = ("prog", T_FULL)
    if key not in _CACHE:
        _CACHE[key] = build_program(T_FULL)
    nc = _CACHE[key]

    in_maps = prep_host_inputs(
        np.asarray(inputs["latent_vec"]), np.asarray(inputs["w_ih0"]),
        np.asarray(inputs["w_hh0"]), np.asarray(inputs["b_ih0"]),
        np.asarray(inputs["b_hh0"]), np.asarray(inputs["w_ih_r"]),
        np.asarray(inputs["w_hh_r"]), np.asarray(inputs["b_ih_r"]),
        np.asarray(inputs["b_hh_r"]), np.asarray(inputs["w_fc"]),
        np.asarray(inputs["b_fc"]))

    res = bass_utils.run_bass_kernel_spmd(nc, in_maps, list(range(N_CORES)))
    out = np.concatenate([res.results[c]["out"] for c in range(N_CORES)], axis=0)
    return out.astype(np.float32)
